# revision 17
# baseline (speedup 1.0000x reference)
"""Trainium2 Bass kernel for nn_CGLayer (gnn_message_passing).

Contract: kernel(**inputs) takes FULL inputs (as reference.setup_inputs()),
returns FULL output [8,128,1,16,9] f32. Data-parallel over batch across 8
NeuronCores; one batch element per core.

Algebraic reduction (exact):
  X   = conn @ vertices                  (message passing)
  Y   = mix_nl(cg(X, X))                 (per-node quadratic in X)
  S   = sum_j sph[:, j, :]               (neighbor sum commutes through the
  Z   = mix_rel(cg(Y, S))                 relative-CG stage)
  out = Z / sqrt(sum Z^2 / 16)

Device pipeline per core — transposed (feature-on-partition) layout with
host-side channel replication; NO PE transposes:
  A:  XAB[cd, lm, t, i]  36 matmuls lhsT=vrep chunks (host-replicated vcat
      columns), rhs=connT; PSUM->SBUF copies cast to fp16.
  P:  P_T[cd, blk, i]    26 fp16 DVE/GpSimd tensor_tensor ops (pair
      products, folded 39-block layout: (l1,l2) unordered, m1<=m2 for
      l1==l2; weights folded host-side).
  Y:  y[a, i]            78 fp16 matmuls lhsT=W2 chunks (K=128 slots,
      M=ncol_g<=48), PSUM-accumulated per s-group; a = packed (g,l,c').
  S:  ssum=reduce_j(sph); S=PE-transpose; SS[128,9,128]=9 sel9 matmuls
      (partition replication of S).
  C:  p2 = y (fp16) * SS  (2 DVE ops); z_T[i,e] = 18 matmuls
      lhsT=p2[:,n,:] (K=a), rhs=W3[a,n,e] (N=144), single PSUM tile.
Host epilogue: gather, unpack e=(l,c',k), global normalization per l.
"""
import numpy as np
from math import factorial, sqrt

MAXL = 2
CH = 16
NN = 128
NB = 8
LDIM = [1, 3, 5]
FOFF = [0, 16, 64]
LMOFF = [0, 1, 4]
SG_NCOL = [16, 32, 48, 32, 16]
# padded a-layout: g-blocks at quadrant-aligned offsets (HW partition-base
# rule); a in [0,112) -> "hi" tile, [128,176) -> "lo" tile.
YOFF = [0, 32, 64, 128, 160, 176]
A_TOT = 176
KOFF = [0, 1, 4]

# ------------------------------------------------------------- CG tables
def _cg_coeff(j1, m1, j2, m2, j3, m3):
    if m3 != m1 + m2:
        return 0.0
    pre = sqrt((2 * j3 + 1) * factorial(j3 + j1 - j2) * factorial(j3 - j1 + j2)
               * factorial(j1 + j2 - j3) / factorial(j1 + j2 + j3 + 1))
    pre *= sqrt(factorial(j3 + m3) * factorial(j3 - m3) * factorial(j1 - m1)
                * factorial(j1 + m1) * factorial(j2 - m2) * factorial(j2 + m2))
    s = 0.0
    vmin = max(0, j2 - j3 - m1, j1 - j3 + m2)
    vmax = min(j1 + j2 - j3, j1 - m1, j2 + m2)
    for v in range(vmin, vmax + 1):
        s += (-1) ** v / (factorial(v) * factorial(j1 + j2 - j3 - v)
                          * factorial(j1 - m1 - v) * factorial(j2 + m2 - v)
                          * factorial(j3 - j2 + m1 + v) * factorial(j3 - j1 - m2 + v))
    return pre * s


def _cg_matrix(l1, l2, l):
    M = np.zeros((2 * l1 + 1, 2 * l2 + 1, 2 * l + 1))
    for m1 in range(-l1, l1 + 1):
        for m2 in range(-l2, l2 + 1):
            if -l <= m1 + m2 <= l:
                M[m1 + l1, m2 + l2, m1 + m2 + l] = _cg_coeff(l1, m1, l2, m2, l, m1 + m2)
    return M


def _valid_pairs(l):
    return [(l1, l2) for l1 in range(3) for l2 in range(3)
            if abs(l1 - l2) <= l <= l1 + l2]


def _lblock(g, l):
    st = g - 2
    return 16 * sum(1 for lp in range(l) if abs(st) <= lp)


def _acol(g, l, cp):
    return YOFF[g] + _lblock(g, l) + cp

# ---- folded block list: (l1,l2,m1,m2), l1<=l2, (l1<l2 or m1<=m2), |st|<=2
def _make_blocks():
    blocks = []
    for l1 in range(3):
        for l2 in range(l1, 3):
            for m1 in range(2 * l1 + 1):
                for m2 in range(2 * l2 + 1):
                    if l1 == l2 and m2 < m1:
                        continue
                    if abs((m1 - l1) + (m2 - l2)) > 2:
                        continue
                    blocks.append((l1, l2, m1, m2))
    return blocks

BLOCKS = _make_blocks()                      # 39
NBLK = len(BLOCKS)
BIDX = {b: i for i, b in enumerate(BLOCKS)}
BLK_G = [(m1 - l1) + (m2 - l2) + 2 for (l1, l2, m1, m2) in BLOCKS]
BLK_NCOL = [SG_NCOL[g] for g in BLK_G]
# w2 sbuf column offsets per (block, half)
W2OFF = np.concatenate([[0], np.cumsum(np.repeat(BLK_NCOL, 2))])
W2COLS = int(W2OFF[-1])

# product op groups: runs of consecutive m2 per (l1, l2, m1)
def _make_qruns():
    runs = []
    i = 0
    while i < NBLK:
        l1, l2, m1, m2 = BLOCKS[i]
        j = i
        while (j + 1 < NBLK and BLOCKS[j + 1][:3] == (l1, l2, m1)
               and BLOCKS[j + 1][3] == BLOCKS[j][3] + 1):
            j += 1
        runs.append((i, l1, l2, m1, BLOCKS[i][3], j - i + 1))
        i = j + 1
    return runs

QRUNS = _make_qruns()
USED_LM_A = sorted({LMOFF[l1] + m1 for (l1, l2, m1, m2) in BLOCKS})
USED_LM_B = sorted({LMOFF[l2] + m2 for (l1, l2, m1, m2) in BLOCKS})

# ------------------------------------------------- host weight assembly
def _assemble_W2n(w_nl):
    """W2n[NBLK, 256, 144] f64: combined CG x w_nl, folded-block layout."""
    W2 = np.zeros((NBLK, 256, A_TOT))
    car, dar = np.meshgrid(np.arange(16), np.arange(16), indexing="ij")
    for l in range(3):
        off = 0
        wl = np.asarray(w_nl[l], np.float64)
        for (p1, p2) in _valid_pairs(l):
            Cg = _cg_matrix(p1, p2, l)
            for m1 in range(2 * p1 + 1):
                for m2 in range(2 * p2 + 1):
                    st = (m1 - p1) + (m2 - p2)
                    if abs(st) > l:
                        continue
                    gc = Cg[m1, m2, st + l]
                    if gc == 0.0:
                        continue
                    g = st + 2
                    if (p1 < p2) or (p1 == p2 and m1 <= m2):
                        bi = BIDX[(p1, p2, m1, m2)]
                        slots = car * 16 + dar
                    else:
                        bi = BIDX[(p2, p1, m2, m1)]
                        slots = dar * 16 + car
                    t = off + car * 16 + dar
                    c0 = _acol(g, l, 0)
                    W2[bi, slots.ravel(), c0:c0 + 16] += gc * wl[t.ravel(), :]
            off += 256
    return W2


def _assemble_W3n(w_rel):
    """W3n[A_TOT, 9, 144]: (a, n) -> e; a = padded Y idx, n = sph (l2,m2)."""
    W3 = np.zeros((A_TOT, 9, 144))
    for l in range(3):
        off = 0
        for (p1, p2) in _valid_pairs(l):
            Cg = _cg_matrix(p1, p2, l)
            wr = np.asarray(w_rel[l], np.float64)
            for m1 in range(2 * p1 + 1):
                for m2 in range(2 * p2 + 1):
                    st = (m1 - p1) + (m2 - p2)
                    if abs(st) > l:
                        continue
                    gc = Cg[m1, m2, st + l]
                    if gc == 0.0:
                        continue
                    a0 = _acol((m1 - p1) + 2, p1, 0)
                    n = LMOFF[p2] + m2
                    for cout in range(16):
                        e = FOFF[l] + cout * LDIM[l] + (st + l)
                        W3[a0:a0 + 16, n, e] += gc * wr[off:off + 16, cout]
            off += 16
    return W3

# ------------------------------------------------------------ bass builder
_NC_CACHE = {}


def _build_nc(debug=False):
    import concourse.bacc as bacc
    import concourse.bass as bass
    import concourse.tile as tile
    from concourse import mybir
    from concourse.masks import make_identity

    f32 = mybir.dt.float32
    f16 = mybir.dt.float16
    nc = bacc.Bacc()
    d_connT = nc.declare_dram_parameter("connT", [128, 128], f16, isOutput=False)
    d_vrep = nc.declare_dram_parameter("vrep", [128, 9 * 4 * 128], f16, isOutput=False)
    d_sph = nc.declare_dram_parameter("sph", [128, 9 * 128], f16, isOutput=False)
    d_w2 = nc.declare_dram_parameter("w2", [128, W2COLS], f16, isOutput=False)
    d_w3h = nc.declare_dram_parameter("w3h", [112, 9 * 144], f16, isOutput=False)
    d_w3l = nc.declare_dram_parameter("w3l", [48, 9 * 144], f16, isOutput=False)
    d_sel9 = nc.declare_dram_parameter("sel9", [9, 9 * 128], f16, isOutput=False)
    d_zout = nc.declare_dram_parameter("zout", [128, 144], f32, isOutput=True)
    if debug:
        d_dxab = nc.declare_dram_parameter("dxab", [128, 9 * 4 * 128], f16, isOutput=True)
        d_dpt = nc.declare_dram_parameter("dpt", [128, NBLK * 2 * 128], f16, isOutput=True)
        d_dyh = nc.declare_dram_parameter("dyh", [112, 128], f16, isOutput=True)
        d_dyl = nc.declare_dram_parameter("dyl", [48, 128], f16, isOutput=True)
        d_dss = nc.declare_dram_parameter("dss", [128, 9 * 128], f16, isOutput=True)

    def vap(t, doff, freedims):
        base = t[:] if not isinstance(t, bass.AP) else t
        return bass.AP(tensor=base.tensor, offset=base.offset + doff,
                       ap=[list(base.ap[0])] + [list(d) for d in freedims])

    with tile.TileContext(nc) as tc:
      with (
        tc.tile_pool(name="sb", bufs=1) as sb,
        tc.tile_pool(name="ps_a", bufs=2, space="PSUM") as ps_a,
        tc.tile_pool(name="ps_y", bufs=1, space="PSUM") as ps_y,
        tc.tile_pool(name="ps_ss", bufs=1, space="PSUM") as ps_ss,
        tc.tile_pool(name="ps_z", bufs=1, space="PSUM") as ps_z,
      ):
        # ---- input DMAs
        connT = sb.tile([128, 128], f16)
        nc.sync.dma_start(out=connT, in_=d_connT[:, :])
        vrep = sb.tile([128, 9, 4, 128], f16)
        for q in range(3):
            nc.sync.dma_start(
                out=vrep[:, 3 * q:3 * (q + 1), :, :],
                in_=d_vrep[:, 1536 * q:1536 * (q + 1)].rearrange(
                    "p (l t i) -> p l t i", l=3, t=4))
        w2 = sb.tile([128, W2COLS], f16)
        h2 = W2COLS // 2
        nc.sync.dma_start(out=w2[:, 0:h2], in_=d_w2[:, 0:h2])
        nc.sync.dma_start(out=w2[:, h2:W2COLS], in_=d_w2[:, h2:W2COLS])
        sph = sb.tile([128, 9, 128], f16)
        nc.sync.dma_start(
            out=sph, in_=d_sph[:, :].rearrange("p (n j) -> p n j", n=9))
        w3h = sb.tile([112, 9, 144], f16)
        nc.sync.dma_start(
            out=w3h, in_=d_w3h[:, :].rearrange("p (n e) -> p n e", n=9))
        w3l = sb.tile([48, 9, 144], f16)
        nc.sync.dma_start(
            out=w3l, in_=d_w3l[:, :].rearrange("p (n e) -> p n e", n=9))
        sel9 = sb.tile([9, 9, 128], f16)
        nc.sync.dma_start(
            out=sel9, in_=d_sel9[:, :].rearrange("p (n i) -> p n i", n=9))
        ident = sb.tile([128, 128], f32)
        make_identity(nc, ident)

        # ---- stage A: XAB[cd, lm, t, i]; t in {A0,A1,B0,B1}
        XAB = sb.tile([128, 9, 4, 128], f16)
        cp_engines = [nc.scalar, nc.vector]
        for lm in range(9):
            t_lo = 0 if lm in USED_LM_A else 2
            t_hi = 4 if lm in USED_LM_B else 2
            if t_lo >= t_hi:
                continue
            a_ps = ps_a.tile([128, 512], f32)
            for t in range(t_lo, t_hi):
                nc.tensor.matmul(a_ps[:, 128 * t:128 * (t + 1)],
                                 vrep[:, lm, t, :], connT, start=True, stop=True)
            nel = (t_hi - t_lo) * 128
            dst = vap(XAB, lm * 512 + t_lo * 128, [[1, nel]])
            src = vap(a_ps, t_lo * 128, [[1, nel]])
            if lm % 2 == 0:
                nc.scalar.activation(dst, src, mybir.ActivationFunctionType.Copy)
            else:
                nc.vector.tensor_copy(out=dst, in_=src)

        # ---- products: P_T[cd, (blk, h), i]  fp16
        PT = sb.tile([128, NBLK, 2, 128], f16)
        for (b0, l1, l2, m1, m2lo, nm2) in QRUNS:
            lm1 = LMOFF[l1] + m1
            lm2 = LMOFF[l2] + m2lo
            for h in range(2):
                eng = nc.vector if h == 0 else nc.gpsimd
                eng.tensor_tensor(
                    out=vap(PT, (b0 * 2 + h) * 128, [[256, nm2], [1, 128]]),
                    in0=vap(XAB, (lm1 * 4 + h) * 128, [[0, nm2], [1, 128]]),
                    in1=vap(XAB, (lm2 * 4 + 2 + h) * 128, [[512, nm2], [1, 128]]),
                    op=mybir.AluOpType.mult)

        # ---- mix: y accumulated per s-group (PE out base must be 0);
        # g0..3 share one bank-tile at col offsets, g4 has its own.
        y_ps = ps_y.tile([128, 512], f32)
        y4_ps = ps_y.tile([16, 128], f32)
        y_tiles = [y_ps[0:SG_NCOL[g], 128 * g:128 * (g + 1)] for g in range(4)]
        y_tiles.append(y4_ps)
        seen = [0] * 5
        total = [sum(1 for g in BLK_G if g == gg) * 2 for gg in range(5)]
        for b in range(NBLK):
            g = BLK_G[b]
            ncol = BLK_NCOL[b]
            for h in range(2):
                seen[g] += 1
                nc.tensor.matmul(
                    y_tiles[g], w2[:, W2OFF[2 * b + h]:W2OFF[2 * b + h] + ncol],
                    PT[:, b, h, :],
                    start=(seen[g] == 1), stop=(seen[g] == total[g]),
                    skip_group_check=True)
        yh_sb = sb.tile([112, 128], f16)
        nc.gpsimd.memset(yh_sb, 0.0)
        yl_sb = sb.tile([48, 128], f16)
        for g in range(5):
            if g < 3:
                dst = yh_sb[YOFF[g]:YOFF[g] + SG_NCOL[g], :]
            else:
                dst = yl_sb[YOFF[g] - 128:YOFF[g] - 128 + SG_NCOL[g], :]
            if g % 2 == 0:
                nc.scalar.activation(dst, y_tiles[g],
                                     mybir.ActivationFunctionType.Copy)
            else:
                nc.vector.tensor_copy(out=dst, in_=y_tiles[g])

        # ---- S: reduce_j sph -> transpose -> replicate across partitions
        z_ps = ps_z.tile([128, 144], f32)
        ssum = sb.tile([128, 9], f32)
        nc.vector.tensor_reduce(
            ssum, sph[:], mybir.AxisListType.X, mybir.AluOpType.add)
        s_ps = z_ps[0:9, 0:128]
        nc.tensor.transpose(s_ps, ssum, ident)
        S16 = sb.tile([9, 128], f16)
        nc.scalar.activation(S16, s_ps, mybir.ActivationFunctionType.Copy)
        SS = sb.tile([128, 9, 128], f16)
        ss_tiles = [ps_ss.tile([128, 512], f32, name="ss_a"),
                    ps_ss.tile([128, 512], f32, name="ss_b"),
                    ps_ss.tile([128, 128], f32, name="ss_c")]
        for n in range(9):
            tn, off = (n // 4, (n % 4) * 128) if n < 8 else (2, 0)
            nc.tensor.matmul(ss_tiles[tn][:, off:off + 128], sel9[:, n, :], S16,
                             start=True, stop=True)
        nc.scalar.activation(vap(SS, 0, [[1, 512]]),
                             vap(ss_tiles[0], 0, [[1, 512]]),
                             mybir.ActivationFunctionType.Copy)
        nc.vector.tensor_copy(out=vap(SS, 512, [[1, 512]]),
                              in_=vap(ss_tiles[1], 0, [[1, 512]]))
        nc.scalar.activation(vap(SS, 1024, [[1, 128]]),
                             vap(ss_tiles[2], 0, [[1, 128]]),
                             mybir.ActivationFunctionType.Copy)

        # ---- stage C: p2 = y * SS; z_T[i, e] = sum_n p2_n.T @ W3_n
        p2h = sb.tile([112, 9, 128], f16)
        nc.vector.tensor_tensor(
            out=p2h, in0=vap(yh_sb, 0, [[0, 9], [1, 128]]),
            in1=SS[0:112, :, :], op=mybir.AluOpType.mult)
        p2l = sb.tile([48, 9, 128], f16)
        nc.gpsimd.tensor_tensor(
            out=p2l, in0=vap(yl_sb, 0, [[0, 9], [1, 128]]),
            in1=SS[0:48, :, :], op=mybir.AluOpType.mult)
        for n in range(9):
            nc.tensor.matmul(z_ps, p2h[:, n, :], w3h[:, n, :],
                             start=(n == 0), stop=False, skip_group_check=True)
            nc.tensor.matmul(z_ps, p2l[:, n, :], w3l[:, n, :],
                             start=False, stop=(n == 8), skip_group_check=True)
        if debug:
            nc.sync.dma_start(out=d_dxab[:, :], in_=XAB)
            for q in range(4):
                o = (NBLK * 2 * 128 // 4) * q
                e = (NBLK * 2 * 128 // 4) * (q + 1)
                nc.sync.dma_start(out=d_dpt[:, o:e], in_=vap(PT, o, [[1, e - o]]))
            nc.sync.dma_start(out=d_dyh[:, :], in_=yh_sb)
            nc.sync.dma_start(out=d_dyl[:, :], in_=yl_sb)
            nc.sync.dma_start(out=d_dss[:, :], in_=SS)
        z_sb = sb.tile([128, 144], f32)
        nc.scalar.activation(z_sb, z_ps, mybir.ActivationFunctionType.Copy)
        nc.sync.dma_start(out=d_zout[:, :], in_=z_sb)

    nc.compile()
    return nc

# ------------------------------------------------------------- host entry
def _get_nc():
    if "nc" not in _NC_CACHE:
        _NC_CACHE["nc"] = _build_nc()
    return _NC_CACHE["nc"]


def kernel(vertices_0, vertices_1, vertices_2, connectivity,
           sph_0, sph_1, sph_2,
           w_nl_0, w_nl_1, w_nl_2,
           w_rel_0, w_rel_1, w_rel_2):
    from concourse.bass_utils import run_bass_kernel_spmd

    f16 = np.float16
    verts = [np.asarray(v) for v in (vertices_0, vertices_1, vertices_2)]
    sphs = [np.asarray(s) for s in (sph_0, sph_1, sph_2)]
    conn = np.asarray(connectivity)

    W2n = _assemble_W2n([w_nl_0, w_nl_1, w_nl_2])
    W3n = _assemble_W3n([w_rel_0, w_rel_1, w_rel_2])
    # w2 sbuf layout: [128 cd-half rows, per-(blk,half) ncol_g columns]
    w2p = np.zeros((128, W2COLS), f16)
    for b in range(NBLK):
        g = BLK_G[b]
        ncol = BLK_NCOL[b]
        blkcols = W2n[b, :, YOFF[g]:YOFF[g] + ncol]
        for h in range(2):
            o = W2OFF[2 * b + h]
            w2p[:, o:o + ncol] = blkcols[h * 128:(h + 1) * 128]
    w3h = np.ascontiguousarray(W3n[0:112].reshape(112, 9 * 144), dtype=f16)
    w3l = np.ascontiguousarray(W3n[128:176].reshape(48, 9 * 144), dtype=f16)
    sel9 = np.zeros((9, 9, 128), f16)
    for n in range(9):
        sel9[n, n, :] = 1.0
    sel9 = sel9.reshape(9, 9 * 128)

    # vrep: host-replicated vcat columns [j, (lm, t, p)]
    cd = np.arange(256)
    in_maps = []
    for b in range(NB):
        vcat = np.concatenate([verts[l][b].reshape(NN, -1) for l in range(3)],
                              axis=1)                       # [j, 144] f32
        vrep = np.zeros((128, 9, 4, 128), f16)
        for l in range(3):
            for m in range(2 * l + 1):
                lm = LMOFF[l] + m
                colA = FOFF[l] + (cd >> 4) * LDIM[l] + m    # c-major (A)
                colB = FOFF[l] + (cd & 15) * LDIM[l] + m    # d-cycling (B)
                vrep[:, lm, 0:2, :] = vcat[:, colA].reshape(128, 2, 128)
                vrep[:, lm, 2:4, :] = vcat[:, colB].reshape(128, 2, 128)
        sph_cat = np.stack([sphs[l][b][:, :, 0, m]
                            for l in range(3) for m in range(2 * l + 1)],
                           axis=1)                          # [i, n, j]
        in_maps.append(dict(
            connT=np.ascontiguousarray(conn[b].T.astype(f16)),
            vrep=np.ascontiguousarray(vrep.reshape(128, 9 * 4 * 128)),
            sph=np.ascontiguousarray(sph_cat.astype(f16).reshape(128, 9 * 128)),
            w2=w2p, w3h=w3h, w3l=w3l, sel9=sel9))

    res = run_bass_kernel_spmd(_get_nc(), in_maps, list(range(NB)))
    globals()["LAST_RES"] = res
    Z = np.stack([res.results[b]["zout"] for b in range(NB)])   # [8, 128, 144]

    # host epilogue: unpack e=(l,c',k), global per-l normalization
    out = np.zeros((NB, 128, 1, 16, 9), dtype=np.float32)
    for l in range(3):
        cols = FOFF[l] + np.arange(16)[:, None] * LDIM[l] + np.arange(LDIM[l])
        blk = Z[:, :, cols]                                 # [b, i, c', k]
        nf = np.sum(blk.astype(np.float64) ** 2)
        out[:, :, 0, :, KOFF[l]:KOFF[l] + LDIM[l]] = blk / np.sqrt(nf / CH)
    return out


# revision 18
# speedup vs baseline: 1.1396x; 1.1396x over previous
"""Trainium2 Bass kernel for nn_CGLayer (gnn_message_passing).

Contract: kernel(**inputs) takes FULL inputs (as reference.setup_inputs()),
returns FULL output [8,128,1,16,9] f32. Data-parallel over batch across 8
NeuronCores; one batch element per core.

Algebraic reduction (exact):
  X   = conn @ vertices                  (message passing)
  Y   = mix_nl(cg(X, X))                 (per-node quadratic in X)
  S   = sum_j sph[:, j, :]               (neighbor sum commutes through the
  Z   = mix_rel(cg(Y, S))                 relative-CG stage)
  out = Z / sqrt(sum Z^2 / 16)

Device pipeline per core — transposed (feature-on-partition) layout with
host-side channel replication; NO PE transposes:
  A:  XAB[cd, lm, t, i]  36 matmuls lhsT=vrep chunks (host-replicated vcat
      columns), rhs=connT; PSUM->SBUF copies cast to fp16.
  P:  P_T[cd, blk, i]    26 fp16 DVE/GpSimd tensor_tensor ops (pair
      products, folded 39-block layout: (l1,l2) unordered, m1<=m2 for
      l1==l2; weights folded host-side).
  Y:  y[a, i]            78 fp16 matmuls lhsT=W2 chunks (K=128 slots,
      M=ncol_g<=48), PSUM-accumulated per s-group; a = packed (g,l,c').
  S:  ssum=reduce_j(sph); S=PE-transpose; SS[128,9,128]=9 sel9 matmuls
      (partition replication of S).
  C:  p2 = y (fp16) * SS  (2 DVE ops); z_T[i,e] = 18 matmuls
      lhsT=p2[:,n,:] (K=a), rhs=W3[a,n,e] (N=144), single PSUM tile.
Host epilogue: gather, unpack e=(l,c',k), global normalization per l.
"""
import numpy as np
from math import factorial, sqrt

MAXL = 2
CH = 16
NN = 128
NB = 8
LDIM = [1, 3, 5]
FOFF = [0, 16, 64]
LMOFF = [0, 1, 4]
SG_NCOL = [16, 32, 48, 32, 16]
# padded a-layout: g-blocks at quadrant-aligned offsets (HW partition-base
# rule); a in [0,112) -> "hi" tile, [128,176) -> "lo" tile.
YOFF = [0, 32, 64, 128, 160, 176]
A_TOT = 176
KOFF = [0, 1, 4]

# ------------------------------------------------------------- CG tables
def _cg_coeff(j1, m1, j2, m2, j3, m3):
    if m3 != m1 + m2:
        return 0.0
    pre = sqrt((2 * j3 + 1) * factorial(j3 + j1 - j2) * factorial(j3 - j1 + j2)
               * factorial(j1 + j2 - j3) / factorial(j1 + j2 + j3 + 1))
    pre *= sqrt(factorial(j3 + m3) * factorial(j3 - m3) * factorial(j1 - m1)
                * factorial(j1 + m1) * factorial(j2 - m2) * factorial(j2 + m2))
    s = 0.0
    vmin = max(0, j2 - j3 - m1, j1 - j3 + m2)
    vmax = min(j1 + j2 - j3, j1 - m1, j2 + m2)
    for v in range(vmin, vmax + 1):
        s += (-1) ** v / (factorial(v) * factorial(j1 + j2 - j3 - v)
                          * factorial(j1 - m1 - v) * factorial(j2 + m2 - v)
                          * factorial(j3 - j2 + m1 + v) * factorial(j3 - j1 - m2 + v))
    return pre * s


def _cg_matrix(l1, l2, l):
    M = np.zeros((2 * l1 + 1, 2 * l2 + 1, 2 * l + 1))
    for m1 in range(-l1, l1 + 1):
        for m2 in range(-l2, l2 + 1):
            if -l <= m1 + m2 <= l:
                M[m1 + l1, m2 + l2, m1 + m2 + l] = _cg_coeff(l1, m1, l2, m2, l, m1 + m2)
    return M


def _valid_pairs(l):
    return [(l1, l2) for l1 in range(3) for l2 in range(3)
            if abs(l1 - l2) <= l <= l1 + l2]


def _lblock(g, l):
    st = g - 2
    return 16 * sum(1 for lp in range(l) if abs(st) <= lp)


def _acol(g, l, cp):
    return YOFF[g] + _lblock(g, l) + cp

# ---- folded block list: (l1,l2,m1,m2), l1<=l2, (l1<l2 or m1<=m2), |st|<=2
def _make_blocks():
    blocks = []
    for l1 in range(3):
        for l2 in range(l1, 3):
            for m1 in range(2 * l1 + 1):
                for m2 in range(2 * l2 + 1):
                    if l1 == l2 and m2 < m1:
                        continue
                    if abs((m1 - l1) + (m2 - l2)) > 2:
                        continue
                    blocks.append((l1, l2, m1, m2))
    return blocks

BLOCKS = _make_blocks()                      # 39
NBLK = len(BLOCKS)
BIDX = {b: i for i, b in enumerate(BLOCKS)}
BLK_G = [(m1 - l1) + (m2 - l2) + 2 for (l1, l2, m1, m2) in BLOCKS]
BLK_NCOL = [SG_NCOL[g] for g in BLK_G]
# w2 sbuf column offsets per (block, half)
W2OFF = np.concatenate([[0], np.cumsum(np.repeat(BLK_NCOL, 2))])
W2COLS = int(W2OFF[-1])

# product op groups: runs of consecutive m2 per (l1, l2, m1)
def _make_qruns():
    runs = []
    i = 0
    while i < NBLK:
        l1, l2, m1, m2 = BLOCKS[i]
        j = i
        while (j + 1 < NBLK and BLOCKS[j + 1][:3] == (l1, l2, m1)
               and BLOCKS[j + 1][3] == BLOCKS[j][3] + 1):
            j += 1
        runs.append((i, l1, l2, m1, BLOCKS[i][3], j - i + 1))
        i = j + 1
    return runs

QRUNS = _make_qruns()
USED_LM_A = sorted({LMOFF[l1] + m1 for (l1, l2, m1, m2) in BLOCKS})
USED_LM_B = sorted({LMOFF[l2] + m2 for (l1, l2, m1, m2) in BLOCKS})

# ------------------------------------------------- host weight assembly
def _assemble_W2n(w_nl):
    """W2n[NBLK, 256, 144] f64: combined CG x w_nl, folded-block layout."""
    W2 = np.zeros((NBLK, 256, A_TOT))
    car, dar = np.meshgrid(np.arange(16), np.arange(16), indexing="ij")
    for l in range(3):
        off = 0
        wl = np.asarray(w_nl[l], np.float64)
        for (p1, p2) in _valid_pairs(l):
            Cg = _cg_matrix(p1, p2, l)
            for m1 in range(2 * p1 + 1):
                for m2 in range(2 * p2 + 1):
                    st = (m1 - p1) + (m2 - p2)
                    if abs(st) > l:
                        continue
                    gc = Cg[m1, m2, st + l]
                    if gc == 0.0:
                        continue
                    g = st + 2
                    if (p1 < p2) or (p1 == p2 and m1 <= m2):
                        bi = BIDX[(p1, p2, m1, m2)]
                        slots = car * 16 + dar
                    else:
                        bi = BIDX[(p2, p1, m2, m1)]
                        slots = dar * 16 + car
                    t = off + car * 16 + dar
                    c0 = _acol(g, l, 0)
                    W2[bi, slots.ravel(), c0:c0 + 16] += gc * wl[t.ravel(), :]
            off += 256
    return W2


def _assemble_W3n(w_rel):
    """W3n[A_TOT, 9, 144]: (a, n) -> e; a = padded Y idx, n = sph (l2,m2)."""
    W3 = np.zeros((A_TOT, 9, 144))
    for l in range(3):
        off = 0
        for (p1, p2) in _valid_pairs(l):
            Cg = _cg_matrix(p1, p2, l)
            wr = np.asarray(w_rel[l], np.float64)
            for m1 in range(2 * p1 + 1):
                for m2 in range(2 * p2 + 1):
                    st = (m1 - p1) + (m2 - p2)
                    if abs(st) > l:
                        continue
                    gc = Cg[m1, m2, st + l]
                    if gc == 0.0:
                        continue
                    a0 = _acol((m1 - p1) + 2, p1, 0)
                    n = LMOFF[p2] + m2
                    for cout in range(16):
                        e = FOFF[l] + cout * LDIM[l] + (st + l)
                        W3[a0:a0 + 16, n, e] += gc * wr[off:off + 16, cout]
            off += 16
    return W3

# ------------------------------------------------------------ bass builder
_NC_CACHE = {}


def _build_nc(debug=False):
    import concourse.bacc as bacc
    import concourse.bass as bass
    import concourse.tile as tile
    from concourse import mybir
    from concourse.masks import make_identity

    f32 = mybir.dt.float32
    f16 = mybir.dt.float16
    nc = bacc.Bacc()
    d_connT = nc.declare_dram_parameter("connT", [128, 128], f16, isOutput=False)
    d_vrep = nc.declare_dram_parameter("vrep", [128, 9 * 4 * 128], f16, isOutput=False)
    d_sph = nc.declare_dram_parameter("sph", [128, 9 * 128], f16, isOutput=False)
    d_w2 = nc.declare_dram_parameter("w2", [128, W2COLS], f16, isOutput=False)
    d_w3h = nc.declare_dram_parameter("w3h", [112, 9 * 144], f16, isOutput=False)
    d_w3l = nc.declare_dram_parameter("w3l", [48, 9 * 144], f16, isOutput=False)
    d_sel9 = nc.declare_dram_parameter("sel9", [9, 9 * 128], f16, isOutput=False)
    d_zout = nc.declare_dram_parameter("zout", [128, 144], f32, isOutput=True)
    if debug:
        d_dxab = nc.declare_dram_parameter("dxab", [128, 9 * 4 * 128], f16, isOutput=True)
        d_dpt = nc.declare_dram_parameter("dpt", [128, NBLK * 2 * 128], f16, isOutput=True)
        d_dyh = nc.declare_dram_parameter("dyh", [112, 128], f16, isOutput=True)
        d_dyl = nc.declare_dram_parameter("dyl", [48, 128], f16, isOutput=True)
        d_dss = nc.declare_dram_parameter("dss", [128, 9 * 128], f16, isOutput=True)

    def vap(t, doff, freedims):
        base = t[:] if not isinstance(t, bass.AP) else t
        return bass.AP(tensor=base.tensor, offset=base.offset + doff,
                       ap=[list(base.ap[0])] + [list(d) for d in freedims])

    with tile.TileContext(nc) as tc:
      with (
        tc.tile_pool(name="sb", bufs=1) as sb,
        tc.tile_pool(name="ps_a", bufs=3, space="PSUM") as ps_a,
        tc.tile_pool(name="ps_y", bufs=1, space="PSUM") as ps_y,
        tc.tile_pool(name="ps_ss", bufs=2, space="PSUM") as ps_ss,
        tc.tile_pool(name="ps_z", bufs=1, space="PSUM") as ps_z,
      ):
        # ---- input DMAs
        connT = sb.tile([128, 128], f16)
        nc.sync.dma_start(out=connT, in_=d_connT[:, :])
        vrep = sb.tile([128, 9, 4, 128], f16)
        for q in range(3):
            nc.sync.dma_start(
                out=vrep[:, 3 * q:3 * (q + 1), :, :],
                in_=d_vrep[:, 1536 * q:1536 * (q + 1)].rearrange(
                    "p (l t i) -> p l t i", l=3, t=4))
        w2 = sb.tile([128, W2COLS], f16)
        h2 = W2COLS // 2
        nc.sync.dma_start(out=w2[:, 0:h2], in_=d_w2[:, 0:h2])
        nc.sync.dma_start(out=w2[:, h2:W2COLS], in_=d_w2[:, h2:W2COLS])
        sph = sb.tile([128, 9, 128], f16)
        nc.sync.dma_start(
            out=sph, in_=d_sph[:, :].rearrange("p (n j) -> p n j", n=9))
        w3h = sb.tile([112, 9, 144], f16)
        nc.sync.dma_start(
            out=w3h, in_=d_w3h[:, :].rearrange("p (n e) -> p n e", n=9))
        w3l = sb.tile([48, 9, 144], f16)
        nc.sync.dma_start(
            out=w3l, in_=d_w3l[:, :].rearrange("p (n e) -> p n e", n=9))
        sel9 = sb.tile([9, 9, 128], f16)
        nc.sync.dma_start(
            out=sel9, in_=d_sel9[:, :].rearrange("p (n i) -> p n i", n=9))
        ident = sb.tile([128, 128], f32)
        make_identity(nc, ident)

        # ---- stage A: XAB[cd, lm, t, i]; t in {A0,A1,B0,B1}
        XAB = sb.tile([128, 9, 4, 128], f16)
        cp_engines = [nc.scalar, nc.vector]
        for lm in range(9):
            t_lo = 0 if lm in USED_LM_A else 2
            t_hi = 4 if lm in USED_LM_B else 2
            if t_lo >= t_hi:
                continue
            a_ps = ps_a.tile([128, 512], f32)
            for t in range(t_lo, t_hi):
                nc.tensor.matmul(a_ps[:, 128 * t:128 * (t + 1)],
                                 vrep[:, lm, t, :], connT, start=True, stop=True)
            nel = (t_hi - t_lo) * 128
            dst = vap(XAB, lm * 512 + t_lo * 128, [[1, nel]])
            src = vap(a_ps, t_lo * 128, [[1, nel]])
            if lm % 2 == 0:
                nc.scalar.activation(dst, src, mybir.ActivationFunctionType.Copy)
            else:
                nc.vector.tensor_copy(out=dst, in_=src)

        # ---- products: P_T[cd, (blk, h), i]  fp16
        PT = sb.tile([128, NBLK, 2, 128], f16)
        runs = sorted(QRUNS, key=lambda r: min(BLK_G[r[0] + k]
                                               for k in range(r[5])))
        for (b0, l1, l2, m1, m2lo, nm2) in runs:
            lm1 = LMOFF[l1] + m1
            lm2 = LMOFF[l2] + m2lo
            for h in range(2):
                eng = nc.vector if h == 0 else nc.gpsimd
                eng.tensor_tensor(
                    out=vap(PT, (b0 * 2 + h) * 128, [[256, nm2], [1, 128]]),
                    in0=vap(XAB, (lm1 * 4 + h) * 128, [[0, nm2], [1, 128]]),
                    in1=vap(XAB, (lm2 * 4 + 2 + h) * 128, [[512, nm2], [1, 128]]),
                    op=mybir.AluOpType.mult)

        # ---- mix: g-major so only one PSUM accumulation group is open at a
        # time (start=True clears has_written for the WHOLE bank; finished
        # values in other column regions survive -- only bits are cleared).
        y_ps = ps_y.tile([128, 512], f32)
        y4_ps = ps_y.tile([16, 512], f32)
        y_tiles = [y_ps[0:SG_NCOL[g], 128 * g:128 * (g + 1)] for g in range(4)]
        y_tiles.append(y4_ps[0:16, 0:128])
        for g in range(5):
            chunks = [(b, h) for b in range(NBLK) if BLK_G[b] == g
                      for h in range(2)]
            ncol = SG_NCOL[g]
            for ci, (b, h) in enumerate(chunks):
                nc.tensor.matmul(
                    y_tiles[g], w2[:, W2OFF[2 * b + h]:W2OFF[2 * b + h] + ncol],
                    PT[:, b, h, :],
                    start=(ci == 0), stop=(ci == len(chunks) - 1))
        yh_sb = sb.tile([112, 128], f16)
        nc.gpsimd.memset(yh_sb, 0.0)
        yl_sb = sb.tile([48, 128], f16)
        for g in range(5):
            if g < 3:
                dst = yh_sb[YOFF[g]:YOFF[g] + SG_NCOL[g], :]
            else:
                dst = yl_sb[YOFF[g] - 128:YOFF[g] - 128 + SG_NCOL[g], :]
            if g % 2 == 0:
                nc.scalar.activation(dst, y_tiles[g],
                                     mybir.ActivationFunctionType.Copy)
            else:
                nc.vector.tensor_copy(out=dst, in_=y_tiles[g])

        # ---- S: reduce_j sph -> transpose -> replicate across partitions
        z_ps = ps_z.tile([128, 144], f32)
        ssum = sb.tile([128, 9], f32)
        nc.vector.tensor_reduce(
            ssum, sph[:], mybir.AxisListType.X, mybir.AluOpType.add)
        s_ps = z_ps[0:9, 0:128]
        nc.tensor.transpose(s_ps, ssum, ident)
        S16 = sb.tile([9, 128], f16)
        nc.scalar.activation(S16, s_ps, mybir.ActivationFunctionType.Copy)
        SS = sb.tile([128, 9, 128], f16)
        for r, n0 in enumerate([0, 4, 8]):
            nr = min(4, 9 - n0)
            ss_ps = ps_ss.tile([128, 512], f32)
            for k in range(nr):
                nc.tensor.matmul(ss_ps[:, 128 * k:128 * (k + 1)],
                                 sel9[:, n0 + k, :], S16,
                                 start=True, stop=True)
            eng_copy = [nc.scalar, nc.vector][r % 2]
            dst = vap(SS, n0 * 128, [[1, nr * 128]])
            srcp = vap(ss_ps, 0, [[1, nr * 128]])
            if eng_copy is nc.scalar:
                nc.scalar.activation(dst, srcp, mybir.ActivationFunctionType.Copy)
            else:
                nc.vector.tensor_copy(out=dst, in_=srcp)

        # ---- stage C: p2 = y * SS; z_T[i, e] = sum_n p2_n.T @ W3_n
        p2h = sb.tile([112, 9, 128], f16)
        nc.vector.tensor_tensor(
            out=p2h, in0=vap(yh_sb, 0, [[0, 9], [1, 128]]),
            in1=SS[0:112, :, :], op=mybir.AluOpType.mult)
        p2l = sb.tile([48, 9, 128], f16)
        nc.gpsimd.tensor_tensor(
            out=p2l, in0=vap(yl_sb, 0, [[0, 9], [1, 128]]),
            in1=SS[0:48, :, :], op=mybir.AluOpType.mult)
        for n in range(9):
            nc.tensor.matmul(z_ps, p2h[:, n, :], w3h[:, n, :],
                             start=(n == 0), stop=False)
            nc.tensor.matmul(z_ps, p2l[:, n, :], w3l[:, n, :],
                             start=False, stop=(n == 8))
        if debug:
            nc.sync.dma_start(out=d_dxab[:, :], in_=XAB)
            for q in range(4):
                o = (NBLK * 2 * 128 // 4) * q
                e = (NBLK * 2 * 128 // 4) * (q + 1)
                nc.sync.dma_start(out=d_dpt[:, o:e], in_=vap(PT, o, [[1, e - o]]))
            nc.sync.dma_start(out=d_dyh[:, :], in_=yh_sb)
            nc.sync.dma_start(out=d_dyl[:, :], in_=yl_sb)
            nc.sync.dma_start(out=d_dss[:, :], in_=SS)
        z_sb = sb.tile([128, 144], f32)
        nc.scalar.activation(z_sb, z_ps, mybir.ActivationFunctionType.Copy)
        nc.sync.dma_start(out=d_zout[:, :], in_=z_sb)

    nc.compile()
    return nc

# ------------------------------------------------------------- host entry
def _get_nc():
    if "nc" not in _NC_CACHE:
        _NC_CACHE["nc"] = _build_nc()
    return _NC_CACHE["nc"]


def kernel(vertices_0, vertices_1, vertices_2, connectivity,
           sph_0, sph_1, sph_2,
           w_nl_0, w_nl_1, w_nl_2,
           w_rel_0, w_rel_1, w_rel_2):
    from concourse.bass_utils import run_bass_kernel_spmd

    f16 = np.float16
    verts = [np.asarray(v) for v in (vertices_0, vertices_1, vertices_2)]
    sphs = [np.asarray(s) for s in (sph_0, sph_1, sph_2)]
    conn = np.asarray(connectivity)

    W2n = _assemble_W2n([w_nl_0, w_nl_1, w_nl_2])
    W3n = _assemble_W3n([w_rel_0, w_rel_1, w_rel_2])
    # w2 sbuf layout: [128 cd-half rows, per-(blk,half) ncol_g columns]
    w2p = np.zeros((128, W2COLS), f16)
    for b in range(NBLK):
        g = BLK_G[b]
        ncol = BLK_NCOL[b]
        blkcols = W2n[b, :, YOFF[g]:YOFF[g] + ncol]
        for h in range(2):
            o = W2OFF[2 * b + h]
            w2p[:, o:o + ncol] = blkcols[h * 128:(h + 1) * 128]
    w3h = np.ascontiguousarray(W3n[0:112].reshape(112, 9 * 144), dtype=f16)
    w3l = np.ascontiguousarray(W3n[128:176].reshape(48, 9 * 144), dtype=f16)
    sel9 = np.zeros((9, 9, 128), f16)
    for n in range(9):
        sel9[n, n, :] = 1.0
    sel9 = sel9.reshape(9, 9 * 128)

    # vrep: host-replicated vcat columns [j, (lm, t, p)]
    cd = np.arange(256)
    in_maps = []
    for b in range(NB):
        vcat = np.concatenate([verts[l][b].reshape(NN, -1) for l in range(3)],
                              axis=1)                       # [j, 144] f32
        vrep = np.zeros((128, 9, 4, 128), f16)
        for l in range(3):
            for m in range(2 * l + 1):
                lm = LMOFF[l] + m
                colA = FOFF[l] + (cd >> 4) * LDIM[l] + m    # c-major (A)
                colB = FOFF[l] + (cd & 15) * LDIM[l] + m    # d-cycling (B)
                vrep[:, lm, 0:2, :] = vcat[:, colA].reshape(128, 2, 128)
                vrep[:, lm, 2:4, :] = vcat[:, colB].reshape(128, 2, 128)
        sph_cat = np.stack([sphs[l][b][:, :, 0, m]
                            for l in range(3) for m in range(2 * l + 1)],
                           axis=1)                          # [i, n, j]
        in_maps.append(dict(
            connT=np.ascontiguousarray(conn[b].T.astype(f16)),
            vrep=np.ascontiguousarray(vrep.reshape(128, 9 * 4 * 128)),
            sph=np.ascontiguousarray(sph_cat.astype(f16).reshape(128, 9 * 128)),
            w2=w2p, w3h=w3h, w3l=w3l, sel9=sel9))

    res = run_bass_kernel_spmd(_get_nc(), in_maps, list(range(NB)))
    globals()["LAST_RES"] = res
    Z = np.stack([res.results[b]["zout"] for b in range(NB)])   # [8, 128, 144]

    # host epilogue: unpack e=(l,c',k), global per-l normalization
    out = np.zeros((NB, 128, 1, 16, 9), dtype=np.float32)
    for l in range(3):
        cols = FOFF[l] + np.arange(16)[:, None] * LDIM[l] + np.arange(LDIM[l])
        blk = Z[:, :, cols]                                 # [b, i, c', k]
        nf = np.sum(blk.astype(np.float64) ** 2)
        out[:, :, 0, :, KOFF[l]:KOFF[l] + LDIM[l]] = blk / np.sqrt(nf / CH)
    return out
